# revision 12
# baseline (speedup 1.0000x reference)
"""Trainium2 Bass kernel for BPNet-style losses (multinomial NLL + count MSE).

Math (per batch sample b, with logits p = pred_prof[b] and counts x = target_prof[b],
both flattened to M = T*L elements):

    log_prob_b = lgamma(n_b+1) - sum_i lgamma(x_bi+1) + sum_i x_bi * logp_bi
    logp = p - logZ_b,  logZ_b = log(sum_i exp(p_bi))     (max-shift not needed:
                                                           |p| <~ 5.5 in f32)
    =>  log_prob_b = lgamma(n_b+1) - SL_b + SXP_b - n_b * logZ_b

with per-sample partial sums the device computes:
    SE_b  = sum exp(p)
    SXP_b = sum x*p
    n_b   = sum x
    SL_b  = sum lgamma(x+1)

x is integer-valued in {0..4}, so lgamma(x+1) is evaluated EXACTLY by a
2-exponential + linear fit (solved offline, residual ~1e-15):
    lgamma(x+1) = C1*2^x + C2*T2^x + A*x + Bc   for x in {0,1,2,3,4}
which turns SL into two extra exp-with-accumulate passes on the scalar engine
plus terms linear in n_b and M.

Sharding: pure data parallel over the batch dim, 32 samples per core x 8 cores.
Each core's [32, 4, L] shard is viewed as [128, L] (partition = sample*4 + task),
the free axis is chunked, and each engine does one pass per chunk:
  ACT: exp(p), exp(ln2*x), exp(lnT2*x)   (each with fused per-partition accum)
  DVE: tensor_tensor_reduce x*p -> SXP,  tensor_scalar x -> n
The per-(sample,task) partials [128, 6] go back to the host, which does the
O(B) scalar combine in f64 (lgamma, log, means).
"""

import math
import sys

for _p in ("/opt/trn_rl_repo",):
    if _p not in sys.path:
        sys.path.insert(0, _p)

import numpy as np

import concourse.bass as bass
import concourse.tile as tile
from concourse import mybir
from concourse.bass_utils import run_bass_kernel_spmd
def _split_multi_waits(nc):
    """The walrus build in this container rejects instructions carrying more
    than one sync-wait ("Too many sync wait commands").  Tile attaches several
    waits to one instruction (kernel-tail drain, multi-input ops).  Move the
    extra waits onto single-wait NoOps spliced immediately before the victim
    on the same engine — per-engine program order makes this equivalent."""
    fn = nc.m.functions[0]
    for blk in fn.blocks:
        insts = blk.instructions
        out = []
        changed = False
        for inst in insts:
            si = inst.sync_info
            waits = list(si.on_wait) if si and si.on_wait else []
            if len(waits) > 1:
                changed = True
                for w in waits[:-1]:
                    nop = mybir.InstNoOp(name=nc.get_next_instruction_name())
                    nop.engine = inst.engine
                    nop.sync_info = mybir.SyncInfo(on_wait=[w], on_update=[])
                    nc.inst_map[nop.name] = nop
                    out.append(nop)
                si.on_wait = [waits[-1]]
                inst.sync_info = si
            out.append(inst)
        if changed:
            blk.instructions = out

N_CORES = 8
B, T, L = 256, 4, 16384
SB = B // N_CORES          # samples per core
P = SB * T                 # 128 partitions = (sample, task)
M_PER_SAMPLE = T * L       # elements per sample

# lgamma(x+1) == C1*T1**x + C2*T2**x + A_LIN*x + B_CONST exactly at x = 0..4
T1 = 2.0
T2 = 0.533475263057
C1 = 0.024335241488
C2 = 3.072944126667
A_LIN = 1.409269208845
B_CONST = -3.097279368155
LN_T1 = math.log(T1)
LN_T2 = math.log(T2)

F32 = mybir.dt.float32
AF = mybir.ActivationFunctionType
ALU = mybir.AluOpType

# Output columns of the per-core [P, 6] partials tensor
COL_SE, COL_S1, COL_S2, COL_SXP, COL_N, COL_DC = range(6)

LAST_RESULTS = None    # BassKernelResults of the most recent run


def build_program(free=L, fc=4096, repeat=1):
    """Build the SPMD single-core Bass program (same program on all cores).

    repeat > 1 re-runs the whole streaming loop over the same inputs that many
    times (benchmark-only: lets wall-clock measurements amortize dispatch and
    transfer overhead via the slope over `repeat`)."""
    nch = free // fc
    assert nch * fc == free

    nc = bass.Bass("TRN2", debug=False, num_devices=N_CORES)
    p_d = nc.dram_tensor("p", [P, free], F32, kind="ExternalInput").ap()
    x_d = nc.dram_tensor("x", [P, free], F32, kind="ExternalInput").ap()
    pc_d = nc.dram_tensor("pc", [P, 1], F32, kind="ExternalInput").ap()
    tc_d = nc.dram_tensor("tc", [P, 1], F32, kind="ExternalInput").ap()
    out_d = nc.dram_tensor("out", [P, 6], F32, kind="ExternalOutput").ap()

    with tile.TileContext(nc) as tc:
        with (
            tc.tile_pool(name="inp", bufs=3) as inp,
            tc.tile_pool(name="scr_a", bufs=1) as scr_a,
            tc.tile_pool(name="scr_v", bufs=1) as scr_v,
            tc.tile_pool(name="acc", bufs=1) as acc,
        ):
            se_sl = acc.tile([P, nch], F32, tag="se")
            s1_sl = acc.tile([P, nch], F32, tag="s1")
            s2_sl = acc.tile([P, nch], F32, tag="s2")
            sxp_sl = acc.tile([P, nch], F32, tag="sxp")
            n_sl = acc.tile([P, nch], F32, tag="n")
            outt = acc.tile([P, 6], F32, tag="outt")
            pc_t = acc.tile([P, 1], F32, tag="pct")
            tc_t = acc.tile([P, 1], F32, tag="tct")

            nc.sync.dma_start(pc_t[:], pc_d[:])
            nc.sync.dma_start(tc_t[:], tc_d[:])

            for c in [c for _ in range(repeat) for c in range(nch)]:
                pt = inp.tile([P, fc], F32, tag="p")
                nc.sync.dma_start(pt[:], p_d[:, c * fc : (c + 1) * fc])
                xt = inp.tile([P, fc], F32, tag="x")
                nc.sync.dma_start(xt[:], x_d[:, c * fc : (c + 1) * fc])

                sa = scr_a.tile([P, fc], F32, tag="sa")
                nc.scalar.activation(
                    sa[:], pt[:], AF.Exp, accum_out=se_sl[:, c : c + 1]
                )
                sa = scr_a.tile([P, fc], F32, tag="sa")
                nc.scalar.activation(
                    sa[:], xt[:], AF.Exp, scale=LN_T1,
                    accum_out=s1_sl[:, c : c + 1],
                )
                sa = scr_a.tile([P, fc], F32, tag="sa")
                nc.scalar.activation(
                    sa[:], xt[:], AF.Exp, scale=LN_T2,
                    accum_out=s2_sl[:, c : c + 1],
                )

                sv = scr_v.tile([P, fc], F32, tag="sv")
                nc.vector.tensor_scalar(
                    sv[:], xt[:], 1.0, None, ALU.mult, ALU.add,
                    accum_out=n_sl[:, c : c + 1],
                )
                sv = scr_v.tile([P, fc], F32, tag="sv")
                nc.vector.scalar_tensor_tensor(
                    sv[:], xt[:], 1.0, pt[:], ALU.mult, ALU.mult,
                    accum_out=sxp_sl[:, c : c + 1],
                )

            for col, sl in (
                (COL_SE, se_sl), (COL_S1, s1_sl), (COL_S2, s2_sl),
                (COL_SXP, sxp_sl), (COL_N, n_sl),
            ):
                nc.vector.reduce_sum(
                    outt[:, col : col + 1], sl[:], axis=mybir.AxisListType.X
                )
            nc.vector.tensor_tensor(
                outt[:, COL_DC : COL_DC + 1], tc_t[:], pc_t[:], ALU.subtract
            )

            nc.sync.dma_start(out_d[:], outt[:])
    _split_multi_waits(nc)
    return nc


_cached_program = None


def _get_program():
    global _cached_program
    if _cached_program is None:
        _cached_program = build_program()
    return _cached_program


def kernel(pred_counts, target_counts, pred_prof, target_prof, count_weights):
    pred_counts = np.asarray(pred_counts, dtype=np.float32)
    target_counts = np.asarray(target_counts, dtype=np.float32)
    pred_prof = np.asarray(pred_prof, dtype=np.float32)
    target_prof = np.asarray(target_prof, dtype=np.float32)
    cw = float(np.asarray(count_weights, dtype=np.float32))

    nc = _get_program()
    in_maps = []
    for i in range(N_CORES):
        s0, s1 = i * SB, (i + 1) * SB
        in_maps.append({
            "p": np.ascontiguousarray(pred_prof[s0:s1].reshape(P, L)),
            "x": np.ascontiguousarray(target_prof[s0:s1].reshape(P, L)),
            "pc": np.ascontiguousarray(pred_counts[s0:s1].reshape(P, 1)),
            "tc": np.ascontiguousarray(target_counts[s0:s1].reshape(P, 1)),
        })

    global LAST_RESULTS
    res = run_bass_kernel_spmd(nc, in_maps, core_ids=list(range(N_CORES)))
    LAST_RESULTS = res

    # Host combine: O(B) scalars in f64.
    nll_sum = 0.0
    sqerr_sum = 0.0
    for i in range(N_CORES):
        out = np.asarray(res.results[i]["out"], dtype=np.float64)  # [P, 6]
        per_sample = out.reshape(SB, T, 6).sum(axis=1)             # [SB, 6]
        se = per_sample[:, COL_SE]
        s1 = per_sample[:, COL_S1]
        s2 = per_sample[:, COL_S2]
        sxp = per_sample[:, COL_SXP]
        n = per_sample[:, COL_N]
        dc = per_sample[:, COL_DC]

        sl = C1 * s1 + C2 * s2 + A_LIN * n + B_CONST * M_PER_SAMPLE
        lgam_n1 = np.array([math.lgamma(v + 1.0) for v in n])
        log_prob = lgam_n1 - sl + sxp - n * np.log(se)
        nll_sum += (-log_prob).sum()
        sqerr_sum += (dc * dc).sum()

    prof_nll = nll_sum / B
    mse = sqerr_sum / B
    return np.asarray(np.float32(prof_nll + cw * mse))
